# revision 1
# baseline (speedup 1.0000x reference)
"""Trainium2 Bass kernel for the Restormer-style channel-attention transformer block.

Full inputs -> shard H-axis over 8 NeuronCores -> single SPMD Bass kernel with an
AllReduce for the channel-attention gram matrices -> gather full output.

Self-contained: all shapes/sharding hardcoded.
"""
import numpy as np

import concourse.bass as bass
import concourse.bacc as bacc
import concourse.tile as tile
import concourse.mybir as mybir
from concourse.bass_utils import run_bass_kernel_spmd

F32 = mybir.dt.float32
F32R = mybir.dt.float32r
BF16 = mybir.dt.bfloat16
AF = mybir.ActivationFunctionType
ALU = mybir.AluOpType
X = mybir.AxisListType.X

C = 96
D = 16
H = 96
W = 96
OWN = 12          # owned H rows per core
HS = 16           # slab H rows (OWN + 2 halo each side)
WP = 98           # padded row width
EPS = 1e-5
HID2 = 512        # padded 2*HID
NCORES = 8

# tap order t = (dd, dh, dw) row-major, offsets in {-1, 0, 1}
TAPS = [(dd, dh, dw) for dd in (-1, 0, 1) for dh in (-1, 0, 1) for dw in (-1, 0, 1)]

# dwconv engine split: which parts run as PE fold vs DVE tap chains
A_FOLD_K = True
A_FOLD_V = True
B_FOLD = (True, True, True, True)     # all FFN blocks PE-fold (HW-line: PE mm ~128ns@490 real vs 204 modeled -> DVE still the silicon bottleneck)
GELU_APPROX = False   # exact Gelu LUT (HW-verified); True = tanh-approx for CoreSim


def _chunks(total, size):
    out = []
    c0 = 0
    while c0 < total:
        cn = min(size, total - c0)
        out.append((c0, cn))
        c0 += cn
    return out


def build_module(num_cores=NCORES, d_slices=D, with_collective=True,
                 a_fold_q=False, b_fold=B_FOLD, reduce_mode="collective"):
    nc = bacc.Bacc("TRN2", target_bir_lowering=False, debug=False,
                   num_devices=num_cores)
    DD = d_slices

    # ---- DRAM I/O ----
    x_d = nc.dram_tensor("x", [C, DD, HS, W], F32, kind="ExternalInput")
    wqT_d = nc.dram_tensor("wqT", [C, C], BF16, kind="ExternalInput")
    dwq_d = nc.dram_tensor("dwq", [C, 27], F32, kind="ExternalInput")
    wkf_d = nc.dram_tensor("wkf", [C, 27, C], BF16, kind="ExternalInput")
    wvf_d = nc.dram_tensor("wvf", [C, 27, C], BF16, kind="ExternalInput")
    wpoT_d = nc.dram_tensor("wpoT", [C, C], BF16, kind="ExternalInput")
    winT_d = nc.dram_tensor("winT", [C, HID2], BF16, kind="ExternalInput")
    ffdw_d = nc.dram_tensor("ffdw", [HID2, 27], F32, kind="ExternalInput")
    winf_d = nc.dram_tensor("winf", [C, 27, 256], BF16, kind="ExternalInput")  # fold blocks 0,1
    winf2_d = nc.dram_tensor("winf2", [C, 27, 256], BF16, kind="ExternalInput")  # fold blocks 2,3
    wqf_d = nc.dram_tensor("wqf", [C, 27, C], BF16, kind="ExternalInput")
    woutT_d = nc.dram_tensor("woutT", [256, C], BF16, kind="ExternalInput")
    temp_d = nc.dram_tensor("tempv", [C, 1], F32, kind="ExternalInput")
    mask0_d = nc.dram_tensor("mask0", [C, 1], F32, kind="ExternalInput")
    mask1_d = nc.dram_tensor("mask1", [C, 1], F32, kind="ExternalInput")
    eye_d = nc.dram_tensor("eye96", [C, C], F32, kind="ExternalInput")
    out_d = nc.dram_tensor("out", [C, DD, OWN, W], F32, kind="ExternalOutput")

    LX = HS * WP          # x/y flat length (1568)
    LZ = 14 * WP          # z / v / x1 / y2 frame length (1372)
    LO = OWN * WP         # owned-rows flat length (1176)

    with tile.TileContext(nc) as tc:
        from contextlib import ExitStack
        ctx = ExitStack()
        cpool = ctx.enter_context(tc.tile_pool(name="consts", bufs=1))

        # ---- load constants ----
        def load_const(name, dram, shape, dtype=F32, cast=None):
            t = cpool.tile(shape, dtype, tag=name)
            nc.sync.dma_start(t[:], dram[:])
            if cast is not None:
                tb = cpool.tile(shape, cast, tag=name + "_b")
                nc.vector.tensor_copy(tb[:], t[:])
                return tb
            return t

        wqT_b = load_const("wqT", wqT_d, [C, C], dtype=BF16)
        dwq_s = load_const("dwq", dwq_d, [C, 27])
        wpoT_b = load_const("wpoT", wpoT_d, [C, C], dtype=BF16)
        winT_b = load_const("winT", winT_d, [C, HID2], dtype=BF16)
        ffdw2 = cpool.tile([128, 27], F32, tag="ffdw2")
        ffdw3 = cpool.tile([128, 27], F32, tag="ffdw3")
        nc.sync.dma_start(ffdw2[:], ffdw_d[256:384, :])
        nc.sync.dma_start(ffdw3[:], ffdw_d[384:512, :])
        temp_s = load_const("tempv", temp_d, [C, 1])
        mask0_s = load_const("mask0", mask0_d, [C, 1])
        mask1_s = load_const("mask1", mask1_d, [C, 1])
        eye_s = load_const("eye96", eye_d, [C, C])
        woutT_b0 = cpool.tile([128, C], BF16, tag="woutT0")
        woutT_b1 = cpool.tile([128, C], BF16, tag="woutT1")
        nc.sync.dma_start(woutT_b0[:], woutT_d[0:128, :])
        nc.sync.dma_start(woutT_b1[:], woutT_d[128:256, :])
        # fold weights: bf16 [96, 27*96] / [96, 27*256]
        wkf_b = cpool.tile([C, 27, C], BF16, tag="wkf")
        wvf_b = cpool.tile([C, 27, C], BF16, tag="wvf")
        winf_b = cpool.tile([C, 27, 256], BF16, tag="winf")
        nc.sync.dma_start(wkf_b[:], wkf_d[:])
        nc.sync.dma_start(wvf_b[:], wvf_d[:])
        nc.sync.dma_start(winf_b[:], winf_d[:])
        if a_fold_q:
            wqf_b = cpool.tile([C, 27, C], BF16, tag="wqf")
            nc.sync.dma_start(wqf_b[:], wqf_d[:])
        if b_fold[2] or b_fold[3]:
            winf2_b = cpool.tile([C, 27, 256], BF16, tag="winf2")
            nc.sync.dma_start(winf2_b[:], winf2_d[:])
        ones_b = cpool.tile([C, C], BF16, tag="ones_b")
        nc.vector.memset(ones_b[:], 1.0)
        ones1 = cpool.tile([1, C], F32, tag="ones1")
        nc.vector.memset(ones1[:], 1.0)
        eps_s = cpool.tile([C, 1], F32, tag="eps_s")
        nc.vector.memset(eps_s[:], EPS)

        sqq_acc = cpool.tile([C, DD], F32, tag="sqq_acc")
        sqk_acc = cpool.tile([C, DD], F32, tag="sqk_acc")

        # ---- pools ----
        vpool = ctx.enter_context(tc.tile_pool(name="vpool", bufs=DD))
        ctxA = ExitStack()
        xA = ctxA.enter_context(tc.tile_pool(name="xA", bufs=2))
        sqp = ctxA.enter_context(tc.tile_pool(name="sqp", bufs=2))
        rsp = ctxA.enter_context(tc.tile_pool(name="rsp", bufs=2))
        yp = ctxA.enter_context(tc.tile_pool(name="yp", bufs=3))
        zqp = ctxA.enter_context(tc.tile_pool(name="zqp", bufs=3))
        qtp = ctxA.enter_context(tc.tile_pool(name="qtp", bufs=2))
        qbp = ctxA.enter_context(tc.tile_pool(name="qbp", bufs=2))
        qTp = ctxA.enter_context(tc.tile_pool(name="qTp", bufs=2))
        scr = ctxA.enter_context(tc.tile_pool(name="scr", bufs=2))

        psG = ctxA.enter_context(tc.tile_pool(name="psG", bufs=1, space="PSUM"))
        psA = ctxA.enter_context(tc.tile_pool(name="psA", bufs=4, space="PSUM"))

        G_ps = psG.tile([C, C], F32, tag="G")

        y_tiles, zq_tiles, v_tiles = {}, {}, {}
        gram_first = [True]

        def ln_pass(src_ap, Lf, tag_prefix, pool_s, pool_r, psum_pool, ps_tag="psA"):
            """uncentered biasfree-LN rstd over channels; src [C, Lf] f32 -> rstd bf16 [C, Lf]"""
            xsq = pool_s.tile([C, Lf], BF16, tag=tag_prefix + "sq")
            nc.scalar.activation(xsq[:], src_ap, AF.Square)
            tln = pool_r.tile([C, Lf], BF16, tag=tag_prefix + "ln")
            n_ch = 4
            csz = Lf // n_ch
            for c0, cn in _chunks(Lf, csz):
                q_ps = psum_pool.tile([C, cn], F32, tag=ps_tag)
                nc.tensor.matmul(q_ps[:], ones_b[:], xsq[:, c0:c0 + cn],
                                 start=True, stop=True)
                nc.scalar.activation(tln[:, c0:c0 + cn], q_ps[:], AF.Ln,
                                     bias=eps_s[:], scale=1.0 / C)
            nc.scalar.activation(tln[:], tln[:], AF.Exp, scale=-0.5)
            return tln

        def fold_conv(y_src, wf_b, o_ch, out_row0, out_len, row0_frame, psum_pool,
                      evac):
            """PE-fold dwconv+conv1x1: 27 matmuls per chunk accumulating in psum.
            y_src: dict of y tiles (flat, guard 1); reads at 1+(out_row0+dh)*98+dw+c0.
            evac(c0, cn, psum_tile) consumes each finished chunk."""
            for c0, cn in _chunks(out_len, 490):
                ps = psum_pool.tile([o_ch, cn], F32, tag="psA")
                for t, (dd, dh, dw_) in enumerate(TAPS):
                    yt = y_src[min(max(dcur[0] + dd, 0), DD - 1)]
                    base = 1 + (out_row0 + dh) * WP + dw_ + c0
                    nc.tensor.matmul(ps[:], wf_b[:, t, :], yt[:, base:base + cn],
                                     start=(t == 0), stop=(t == 26))
                evac(c0, cn, ps)

        dcur = [0]

        def a_emit(d):
            dcur[0] = d
            q_bf = qbp.tile([C, OWN, W], BF16, tag="q_bf")
            if a_fold_q:
                def q_evac(c0, cn, ps):
                    rn = cn // WP
                    r0 = c0 // WP
                    p3 = ps[:].rearrange("p (r w) -> p r w", w=WP)
                    nc.scalar.activation(q_bf[:, r0:r0 + rn, :], p3[:, :, 1:97],
                                         AF.Copy)
                fold_conv(y_tiles, wqf_b, C, 2, LO, None, psA, q_evac)
            else:
                # --- q: DVE tap chain on z_q ---
                q_t = qtp.tile([C, LO + 2], BF16, tag="q_t")
                for t, (dd, dh, dw_) in enumerate(TAPS):
                    zt = zq_tiles[min(max(d + dd, 0), DD - 1)]
                    base = 99 + dh * WP + dw_
                    src = zt[:, base:base + LO]
                    if t == 0:
                        nc.vector.tensor_scalar(q_t[:, 0:LO], src, dwq_s[:, 0:1],
                                                None, ALU.mult)
                    else:
                        nc.vector.scalar_tensor_tensor(q_t[:, 0:LO], src,
                                                       dwq_s[:, t:t + 1],
                                                       q_t[:, 0:LO],
                                                       ALU.mult, ALU.add)
                q3 = q_t[:, 0:LO].rearrange("p (r w) -> p r w", w=WP)
                nc.scalar.activation(q_bf[:], q3[:, :, 1:97], AF.Copy)

            # --- k: PE fold (owned rows frame LO) ---
            k_bf = qbp.tile([C, OWN, W], BF16, tag="k_bf")

            def k_evac(c0, cn, ps):
                rn = cn // WP
                r0 = c0 // WP
                p3 = ps[:].rearrange("p (r w) -> p r w", w=WP)
                nc.scalar.activation(k_bf[:, r0:r0 + rn, :], p3[:, :, 1:97], AF.Copy)

            fold_conv(y_tiles, wkf_b, C, 2, LO, None, psA, k_evac)

            # --- v: PE fold (rows 1..14, frame LZ), keep flat bf16 ---
            v_bf = vpool.tile([C, LZ + 3], BF16, tag="v_bf")
            v_tiles[d] = v_bf

            def v_evac(c0, cn, ps):
                nc.scalar.activation(v_bf[:, 1 + c0:1 + c0 + cn], ps[:], AF.Copy)

            fold_conv(y_tiles, wvf_b, C, 1, LZ, None, psA, v_evac)

            # --- sumsq + gram ---
            sc1 = scr.tile([C, OWN * W], BF16, tag="sc1")
            nc.scalar.activation(sc1[:], q_bf[:].rearrange("p r w -> p (r w)"),
                                 AF.Square, accum_out=sqq_acc[:, d:d + 1])
            sc2 = scr.tile([C, OWN * W], BF16, tag="sc2")
            nc.scalar.activation(sc2[:], k_bf[:].rearrange("p r w -> p (r w)"),
                                 AF.Square, accum_out=sqk_acc[:, d:d + 1])
            qT = qTp.tile([128, 9, C], BF16, tag="qT")
            kT = qTp.tile([128, 9, C], BF16, tag="kT")
            nc.sync.dma_start_transpose(qT[:], q_bf[:].rearrange("p r w -> p (r w)"))
            nc.sync.dma_start_transpose(kT[:], k_bf[:].rearrange("p r w -> p (r w)"))
            for j in range(9):
                nc.tensor.matmul(G_ps[:], qT[:, j, :], kT[:, j, :],
                                 start=gram_first[0], stop=(d == DD - 1 and j == 8))
                gram_first[0] = False

        # ================= phase A =================
        for d in range(DD):
            x_t = xA.tile([C, LX + 3], F32, tag="x_t")
            x3 = x_t[:, 1:1 + LX].rearrange("p (r w) -> p r w", w=WP)
            nc.sync.dma_start(x3[:, :, 1:97], x_d[:, d, :, :])
            nc.scalar.activation(x3[:, :, 0:1], x3[:, :, 1:2], AF.Copy)
            nc.scalar.activation(x3[:, :, 97:98], x3[:, :, 96:97], AF.Copy)
            rstd = ln_pass(x_t[:, 1:1 + LX], LX, "A", sqp, rsp, psA)
            y_t = yp.tile([C, LX + 3], BF16, tag="y_t")
            nc.vector.memset(y_t[:, 0:1], 0.0)
            nc.vector.memset(y_t[:, 1 + LX:LX + 3], 0.0)
            nc.vector.tensor_mul(y_t[:, 1:1 + LX], x_t[:, 1:1 + LX], rstd[:])
            y_tiles[d] = y_t
            if not a_fold_q:
                # z_q = wqT^T @ y rows 1..14
                zq = zqp.tile([C, LZ + 3], BF16, tag="zq")
                nc.vector.memset(zq[:, 0:1], 0.0)
                nc.vector.memset(zq[:, 1 + LZ:LZ + 3], 0.0)
                for c0, cn in _chunks(LZ, 490):
                    ps = psA.tile([C, cn], F32, tag="psA")
                    nc.tensor.matmul(ps[:], wqT_b[:],
                                     y_t[:, 1 + WP + c0:1 + WP + c0 + cn],
                                     start=True, stop=True)
                    nc.scalar.activation(zq[:, 1 + c0:1 + c0 + cn], ps[:], AF.Copy)
                zq_tiles[d] = zq
            if d >= 1:
                a_emit(d - 1)
        a_emit(DD - 1)

        # ================= reduce =================
        sqq = cpool.tile([C, 1], F32, tag="sqq")
        sqk = cpool.tile([C, 1], F32, tag="sqk")
        nc.vector.tensor_reduce(sqq[:], sqq_acc[:], X, ALU.add)
        nc.vector.tensor_reduce(sqk[:], sqk_acc[:], X, ALU.add)
        pack = cpool.tile([C, C + 2], F32, tag="pack")
        nc.scalar.activation(pack[:, 0:C], G_ps[:], AF.Copy)
        nc.vector.tensor_copy(pack[:, C:C + 1], sqq[:])
        nc.vector.tensor_copy(pack[:, C + 1:C + 2], sqk[:])

        ctxA.close()
        tot = cpool.tile([C, C + 2], F32, tag="tot")
        if not with_collective:
            nc.vector.tensor_copy(tot[:], pack[:])
        elif reduce_mode == "collective":
            dram = ctx.enter_context(tc.tile_pool(name="dram", bufs=1, space="DRAM"))
            red_in = dram.tile([C, C + 2], F32, tag="red_in")
            red_out = dram.tile([C, C + 2], F32, tag="red_out")
            nc.sync.dma_start(red_in[:], pack[:])
            nc.gpsimd.collective_compute(
                "AllReduce", ALU.add,
                replica_groups=[list(range(num_cores))],
                ins=[red_in.opt()],
                outs=[red_out.opt()],
            )
            nc.sync.dma_start(tot[:], red_out[:])
        else:
            # peer-DMA all-reduce: each core sends its pack to all 7 peers
            # (relative dest Δtpb=j lands in sender-unique slot j), then sums.
            PW = C + 2
            pack128 = cpool.tile([128, PW], F32, tag="pack128")
            slots = cpool.tile([128, (num_cores - 1) * PW], F32, tag="slots")
            acc128 = cpool.tile([128, PW], F32, tag="acc128")
            nc.vector.memset(pack128[96:128, :], 0.0)
            nc.vector.tensor_copy(pack128[0:96, :], pack[:])
            rsem = nc.alloc_semaphore("peer_rsem")
            lsem = nc.alloc_semaphore("peer_lsem")
            with tc.tile_critical():
                g = nc.gpsimd
                for j in range(1, num_cores):
                    rd = [None] * 8
                    rd[j] = (0, j)
                    g.remote_dma_broadcast(slots[:, (j - 1) * PW:j * PW],
                                           pack128[:], remote_sem=rsem,
                                           local_sem=lsem, rdests=rd)
                g.trigger_dma(count=num_cores - 1)
                g.wait_ge(rsem, 2 * (num_cores - 1))
                g.tensor_add(acc128[:], pack128[:], slots[:, 0:PW])
                for j in range(2, num_cores):
                    g.tensor_add(acc128[:], acc128[:],
                                 slots[:, (j - 1) * PW:j * PW])
                g.tensor_copy(tot[:], acc128[0:96, :])

        # ================= attention matrix =================
        psS = ctx.enter_context(tc.tile_pool(name="psS", bufs=1, space="PSUM"))
        nq = cpool.tile([C, 1], F32, tag="nq")
        nk = cpool.tile([C, 1], F32, tag="nk")
        nc.scalar.activation(nq[:], tot[:, C:C + 1], AF.Sqrt)
        nc.scalar.activation(nk[:], tot[:, C + 1:C + 2], AF.Sqrt)
        nc.vector.tensor_scalar_max(nq[:], nq[:], 1e-12)
        nc.vector.tensor_scalar_max(nk[:], nk[:], 1e-12)
        rq = cpool.tile([C, 1], F32, tag="rq")
        rk = cpool.tile([C, 1], F32, tag="rk")
        nc.vector.reciprocal(rq[:], nq[:])
        nc.vector.reciprocal(rk[:], nk[:])
        rqt = cpool.tile([C, 1], F32, tag="rqt")
        nc.vector.tensor_mul(rqt[:], rq[:], temp_s[:])
        rkT_ps = psS.tile([1, C], F32, tag="psS_row")
        nc.tensor.transpose(rkT_ps[:], rk[:], eye_s[:])
        rkT = cpool.tile([1, C], F32, tag="rkT")
        nc.scalar.activation(rkT[:], rkT_ps[:], AF.Copy)
        rk_rep = psS.tile([C, C], F32, tag="psS_rep")
        nc.tensor.matmul(rk_rep[:], ones1[:], rkT[:], start=True, stop=True)
        lg = cpool.tile([C, C], F32, tag="lg")
        nc.vector.scalar_tensor_tensor(lg[:], tot[:, 0:C], rqt[:], rk_rep[:],
                                       ALU.mult, ALU.mult)
        ma = cpool.tile([C, 1], F32, tag="ma")
        mb = cpool.tile([C, 1], F32, tag="mb")
        nc.vector.tensor_reduce(ma[:], lg[:, 0:48], X, ALU.max)
        nc.vector.tensor_reduce(mb[:], lg[:, 48:96], X, ALU.max)
        nc.vector.tensor_scalar_mul(mb[:], mb[:], mask1_s[:])
        mneg = cpool.tile([C, 1], F32, tag="mneg")
        nc.vector.scalar_tensor_tensor(mneg[:], ma[:], mask0_s[:], mb[:],
                                       ALU.mult, ALU.add)
        nc.vector.tensor_scalar_mul(mneg[:], mneg[:], -1.0)
        et = cpool.tile([C, C], F32, tag="et")
        nc.scalar.activation(et[:], lg[:], AF.Exp, bias=mneg[:])
        sa = cpool.tile([C, 1], F32, tag="sa")
        sb = cpool.tile([C, 1], F32, tag="sb")
        nc.vector.tensor_reduce(sa[:], et[:, 0:48], X, ALU.add)
        nc.vector.tensor_reduce(sb[:], et[:, 48:96], X, ALU.add)
        nc.vector.tensor_scalar_mul(sb[:], sb[:], mask1_s[:])
        ssum = cpool.tile([C, 1], F32, tag="ssum")
        nc.vector.scalar_tensor_tensor(ssum[:], sa[:], mask0_s[:], sb[:],
                                       ALU.mult, ALU.add)
        rsum = cpool.tile([C, 1], F32, tag="rsum")
        nc.vector.reciprocal(rsum[:], ssum[:])
        attn = cpool.tile([C, C], F32, tag="attn")
        nc.vector.tensor_scalar(attn[:, 0:48], et[:, 0:48], rsum[:], mask0_s[:],
                                ALU.mult, ALU.mult)
        nc.vector.tensor_scalar(attn[:, 48:96], et[:, 48:96], rsum[:], mask1_s[:],
                                ALU.mult, ALU.mult)
        attnT_ps = psS.tile([C, C], F32, tag="psS_rep")
        nc.tensor.transpose(attnT_ps[:], attn[:], eye_s[:])
        attnT_b = cpool.tile([C, C], BF16, tag="attnT_b")
        nc.scalar.activation(attnT_b[:], attnT_ps[:], AF.Copy)

        # ================= phase B =================
        xB = ctx.enter_context(tc.tile_pool(name="xB", bufs=2))
        oap = ctx.enter_context(tc.tile_pool(name="oap", bufs=2))
        x1p = ctx.enter_context(tc.tile_pool(name="x1p", bufs=3))
        sq2 = ctx.enter_context(tc.tile_pool(name="sq2", bufs=2))
        rs2 = ctx.enter_context(tc.tile_pool(name="rs2", bufs=2))
        y2p = ctx.enter_context(tc.tile_pool(name="y2p", bufs=3))
        zhp = ctx.enter_context(tc.tile_pool(name="zhp", bufs=3))
        hp = ctx.enter_context(tc.tile_pool(name="hp", bufs=2))
        gp = ctx.enter_context(tc.tile_pool(name="gp", bufs=2))
        otp = ctx.enter_context(tc.tile_pool(name="otp", bufs=2))
        psB = ctx.enter_context(tc.tile_pool(name="psB", bufs=4, space="PSUM"))

        x1_tiles, y2_tiles, zh2_tiles, zh3_tiles = {}, {}, {}, {}
        zh_map = {2: zh2_tiles, 3: zh3_tiles}

        def b_emit(d):
            dcur[0] = d
            h_blocks = []
            for b in range(4):
                h_b = hp.tile([128, LO + 2], BF16, tag=f"h{b}")
                if b_fold[b]:
                    wsrc = winf_b if b < 2 else winf2_b
                    wf = wsrc[:, :, 128 * (b % 2):128 * (b % 2 + 1)]
                    for c0, cn in _chunks(LO, 490):
                        ps = psB.tile([128, cn], F32, tag="psB")
                        for t, (dd, dh, dw_) in enumerate(TAPS):
                            yt = y2_tiles[min(max(d + dd, 0), DD - 1)]
                            base = 1 + (1 + dh) * WP + dw_ + c0
                            nc.tensor.matmul(ps[:], wf[:, t, :],
                                             yt[:, base:base + cn],
                                             start=(t == 0), stop=(t == 26))
                        nc.scalar.activation(h_b[:, c0:c0 + cn], ps[:], AF.Copy)
                else:
                    zh_tiles = zh_map[b]
                    dwh = ffdw2 if b == 2 else ffdw3
                    for t, (dd, dh, dw_) in enumerate(TAPS):
                        zt = zh_tiles[min(max(d + dd, 0), DD - 1)]
                        base = 99 + dh * WP + dw_
                        src = zt[:, base:base + LO]
                        if t == 0:
                            nc.vector.tensor_scalar(h_b[:, 0:LO], src, dwh[:, 0:1],
                                                    None, ALU.mult)
                        else:
                            nc.vector.scalar_tensor_tensor(h_b[:, 0:LO], src,
                                                           dwh[:, t:t + 1],
                                                           h_b[:, 0:LO],
                                                           ALU.mult, ALU.add)
                h_blocks.append(h_b)
            # gelu gate: g_i = gelu(h_i) * h_{i+2}
            g_blocks = []
            for i in (0, 1):
                gl = gp.tile([128, LO + 2], BF16, tag=f"gl{i}")
                hin = h_blocks[i][:, 0:LO]
                if not GELU_APPROX:
                    nc.scalar.activation(gl[:, 0:LO], hin, AF.Gelu)
                else:
                    u = gp.tile([128, LO], BF16, tag=f"gu{i}")
                    nc.scalar.activation(u[:], hin, AF.Square)
                    nc.vector.scalar_tensor_tensor(u[:], u[:], 0.044715, hin,
                                                   ALU.mult, ALU.mult)
                    nc.vector.tensor_add(u[:], u[:], hin)
                    nc.scalar.activation(u[:], u[:], AF.Tanh, scale=0.7978845608)
                    nc.vector.scalar_tensor_tensor(u[:], u[:], 0.5, hin,
                                                   ALU.mult, ALU.mult)
                    nc.vector.scalar_tensor_tensor(gl[:, 0:LO], hin, 0.5, u[:],
                                                   ALU.mult, ALU.add)
                nc.vector.tensor_mul(gl[:, 0:LO], gl[:, 0:LO],
                                     h_blocks[2 + i][:, 0:LO])
                g_blocks.append(gl)
            # out conv + residual
            out_t = otp.tile([C, LO + 2], F32, tag="out_t")
            x1 = x1_tiles[d]
            for c0, cn in _chunks(LO, 490):
                ps = psB.tile([C, cn], F32, tag="psB")
                nc.tensor.matmul(ps[:], woutT_b0[:], g_blocks[0][:, c0:c0 + cn],
                                 start=True, stop=False)
                nc.tensor.matmul(ps[:], woutT_b1[:], g_blocks[1][:, c0:c0 + cn],
                                 start=False, stop=True)
                nc.vector.tensor_add(out_t[:, c0:c0 + cn], ps[:],
                                     x1[:, 1 + WP + c0:1 + WP + c0 + cn])
            o3 = out_t[:, 0:LO].rearrange("p (r w) -> p r w", w=WP)
            nc.sync.dma_start(out_d[:, d, :, :], o3[:, :, 1:97])

        for d in range(DD):
            v_bf = v_tiles[d]
            # oa = attnT^T @ v
            oa = oap.tile([C, LZ + 3], BF16, tag="oa")
            for c0, cn in _chunks(LZ, 490):
                ps = psB.tile([C, cn], F32, tag="psB")
                nc.tensor.matmul(ps[:], attnT_b[:], v_bf[:, 1 + c0:1 + c0 + cn],
                                 start=True, stop=True)
                nc.scalar.activation(oa[:, 1 + c0:1 + c0 + cn], ps[:], AF.Copy)
            # x1 = x + wpo^T @ oa
            x2_t = xB.tile([C, LZ + 3], F32, tag="x2_t")
            x23 = x2_t[:, 1:1 + LZ].rearrange("p (r w) -> p r w", w=WP)
            nc.vector.memset(x23[:, :, 0:1], 0.0)
            nc.vector.memset(x23[:, :, 97:98], 0.0)
            nc.sync.dma_start(x23[:, :, 1:97], x_d[:, d, 1:15, :])
            x1 = x1p.tile([C, LZ + 3], F32, tag="x1")
            for c0, cn in _chunks(LZ, 490):
                ps = psB.tile([C, cn], F32, tag="psB")
                nc.tensor.matmul(ps[:], wpoT_b[:], oa[:, 1 + c0:1 + c0 + cn],
                                 start=True, stop=True)
                nc.vector.tensor_add(x1[:, 1 + c0:1 + c0 + cn], ps[:],
                                     x2_t[:, 1 + c0:1 + c0 + cn])
            x13 = x1[:, 1:1 + LZ].rearrange("p (r w) -> p r w", w=WP)
            nc.scalar.activation(x13[:, :, 0:1], x13[:, :, 1:2], AF.Copy)
            nc.scalar.activation(x13[:, :, 97:98], x13[:, :, 96:97], AF.Copy)
            x1_tiles[d] = x1
            # LN2 + y2
            rstd2 = ln_pass(x1[:, 1:1 + LZ], LZ, "B", sq2, rs2, psB, ps_tag="psB")
            y2 = y2p.tile([C, LZ + 3], BF16, tag="y2")
            nc.vector.memset(y2[:, 0:1], 0.0)
            nc.vector.memset(y2[:, 1 + LZ:LZ + 3], 0.0)
            nc.vector.tensor_mul(y2[:, 1:1 + LZ], x1[:, 1:1 + LZ], rstd2[:])
            y2_tiles[d] = y2
            # z_h for DVE blocks
            for b in (2, 3):
                if b_fold[b]:
                    continue
                zh_tiles = zh_map[b]
                zh = zhp.tile([128, LZ + 3], BF16, tag=f"zh{b}")
                nc.vector.memset(zh[:, 0:1], 0.0)
                nc.vector.memset(zh[:, 1 + LZ:LZ + 3], 0.0)
                for c0, cn in _chunks(LZ, 490):
                    ps = psB.tile([128, cn], F32, tag="psB")
                    nc.tensor.matmul(ps[:], winT_b[:, 128 * b:128 * (b + 1)],
                                     y2[:, 1 + c0:1 + c0 + cn], start=True, stop=True)
                    nc.scalar.activation(zh[:, 1 + c0:1 + c0 + cn], ps[:], AF.Copy)
                zh_tiles[d] = zh
            if d >= 1:
                b_emit(d - 1)
        b_emit(DD - 1)

        ctx.close()

    nc.compile()
    return nc


def _prep_weights(inp):
    import ml_dtypes
    bf = ml_dtypes.bfloat16
    w = {}
    ln1 = np.asarray(inp['ln1_w'], np.float32)
    ln2 = np.asarray(inp['ln2_w'], np.float32)
    w_qkv = np.asarray(inp['w_qkv'], np.float32) * ln1[None, :]
    w_dw = np.asarray(inp['w_dw'], np.float32).reshape(288, 27)
    w['wqT'] = np.ascontiguousarray(w_qkv[0:96].T)
    w['dwq'] = np.ascontiguousarray(w_dw[0:96])
    wqT0 = w_qkv[0:96].T
    dwq0 = w_dw[0:96]
    w['wqf'] = np.ascontiguousarray(
        np.stack([wqT0 * dwq0[None, :, t] for t in range(27)]).transpose(1, 0, 2))
    wkT = w_qkv[96:192].T
    dwk = w_dw[96:192]
    w['wkf'] = np.ascontiguousarray(
        np.stack([wkT * dwk[None, :, t] for t in range(27)]).transpose(1, 0, 2))
    wvT = w_qkv[192:288].T
    dwv = w_dw[192:288]
    w['wvf'] = np.ascontiguousarray(
        np.stack([wvT * dwv[None, :, t] for t in range(27)]).transpose(1, 0, 2))
    w['wpoT'] = np.ascontiguousarray(np.asarray(inp['w_po'], np.float32).T)
    w_in = np.asarray(inp['w_in'], np.float32) * ln2[None, :]
    w_in_p = np.zeros((HID2, 96), np.float32)
    w_in_p[0:255] = w_in[0:255]
    w_in_p[256:511] = w_in[255:510]
    w_ffdw = np.asarray(inp['w_ffdw'], np.float32).reshape(510, 27)
    w_ffdw_p = np.zeros((HID2, 27), np.float32)
    w_ffdw_p[0:255] = w_ffdw[0:255]
    w_ffdw_p[256:511] = w_ffdw[255:510]
    w['winT'] = np.ascontiguousarray(w_in_p.T)
    w['ffdw'] = w_ffdw_p
    w['winf'] = np.ascontiguousarray(
        np.stack([w_in_p.T[:, 0:256] * w_ffdw_p[None, 0:256, t].reshape(1, 256)
                  for t in range(27)]).transpose(1, 0, 2))
    w['winf2'] = np.ascontiguousarray(
        np.stack([w_in_p.T[:, 256:512] * w_ffdw_p[None, 256:512, t].reshape(1, 256)
                  for t in range(27)]).transpose(1, 0, 2))
    w_out = np.asarray(inp['w_out'], np.float32)
    w_out_p = np.zeros((96, 256), np.float32)
    w_out_p[:, 0:255] = w_out
    w['woutT'] = np.ascontiguousarray(w_out_p.T)
    w['tempv'] = np.repeat(
        np.asarray(inp['temperature'], np.float32).reshape(2), 48)[:, None].copy()
    mask0 = np.zeros((96, 1), np.float32)
    mask0[0:48] = 1
    w['mask0'] = mask0
    w['mask1'] = np.ascontiguousarray(1 - mask0)
    w['eye96'] = np.eye(96, dtype=np.float32)
    for name in ('wqT', 'wkf', 'wvf', 'wpoT', 'winT', 'winf', 'winf2', 'wqf', 'woutT'):
        w[name] = w[name].astype(bf)
    return w


_CACHE = {}


def kernel(**inputs):
    if 'nc' not in _CACHE:
        _CACHE['nc'] = build_module()
    nc = _CACHE['nc']
    w = _prep_weights(inputs)
    x = np.asarray(inputs['x'], np.float32)[0]     # [96, 16, 96, 96]
    in_maps = []
    for c in range(NCORES):
        h0 = OWN * c
        idx = np.clip(np.arange(h0 - 2, h0 + 14), 0, H - 1)
        m = {'x': np.ascontiguousarray(x[:, :, idx, :])}
        m.update(w)
        in_maps.append(m)
    res = run_bass_kernel_spmd(nc, in_maps, core_ids=list(range(NCORES)))
    out = np.concatenate([res.results[c]['out'] for c in range(NCORES)], axis=2)
    return out[None].astype(np.float32)


if __name__ == "__main__":
    import jax
    out = kernel(**{k: np.asarray(v) for k, v in __import__('reference').setup_inputs().items()})
    print(out.shape, out.dtype)



# revision 3
# speedup vs baseline: 1.5769x; 1.5769x over previous
"""Trainium2 Bass kernel for the Restormer-style channel-attention transformer block.

Full inputs -> shard H-axis over 8 NeuronCores -> single SPMD Bass kernel with an
AllReduce for the channel-attention gram matrices -> gather full output.

Self-contained: all shapes/sharding hardcoded.
"""
import numpy as np

import concourse.bass as bass
import concourse.bacc as bacc
import concourse.tile as tile
import concourse.mybir as mybir
from concourse.bass_utils import run_bass_kernel_spmd

F32 = mybir.dt.float32
F32R = mybir.dt.float32r
BF16 = mybir.dt.bfloat16
AF = mybir.ActivationFunctionType
ALU = mybir.AluOpType
X = mybir.AxisListType.X

C = 96
D = 16
H = 96
W = 96
OWN = 12          # owned H rows per core
HS = 16           # slab H rows (OWN + 2 halo each side)
WP = 98           # padded row width
EPS = 1e-5
HID2 = 512        # padded 2*HID
NCORES = 8

# tap order t = (dd, dh, dw) row-major, offsets in {-1, 0, 1}
TAPS = [(dd, dh, dw) for dd in (-1, 0, 1) for dh in (-1, 0, 1) for dw in (-1, 0, 1)]

# dwconv engine split: which parts run as PE fold vs DVE tap chains
A_FOLD_K = True
A_FOLD_V = True
B_FOLD = (True, True, True, True)     # all FFN blocks PE-fold (HW-line: PE mm ~128ns@490 real vs 204 modeled -> DVE still the silicon bottleneck)
GELU_APPROX = False   # exact Gelu LUT (HW-verified); True = tanh-approx for CoreSim


def _chunks(total, size):
    out = []
    c0 = 0
    while c0 < total:
        cn = min(size, total - c0)
        out.append((c0, cn))
        c0 += cn
    return out


def build_module(num_cores=NCORES, d_slices=D, with_collective=True,
                 a_fold_q=True, b_fold=B_FOLD, reduce_mode="collective",
                 reps=1):
    nc = bacc.Bacc("TRN2", target_bir_lowering=False, debug=False,
                   num_devices=num_cores)
    DD = d_slices

    # ---- DRAM I/O ----
    x_d = nc.dram_tensor("x", [C, DD, HS, W], F32, kind="ExternalInput")
    wqT_d = nc.dram_tensor("wqT", [C, C], BF16, kind="ExternalInput")
    dwq_d = nc.dram_tensor("dwq", [C, 27], F32, kind="ExternalInput")
    wkf_d = nc.dram_tensor("wkf", [C, 27, C], BF16, kind="ExternalInput")
    wvf_d = nc.dram_tensor("wvf", [C, 27, C], BF16, kind="ExternalInput")
    wpoT_d = nc.dram_tensor("wpoT", [C, C], BF16, kind="ExternalInput")
    winT_d = nc.dram_tensor("winT", [C, HID2], BF16, kind="ExternalInput")
    ffdw_d = nc.dram_tensor("ffdw", [HID2, 27], F32, kind="ExternalInput")
    winf_d = nc.dram_tensor("winf", [C, 27, 256], BF16, kind="ExternalInput")  # fold blocks 0,1
    winf2_d = nc.dram_tensor("winf2", [C, 27, 256], BF16, kind="ExternalInput")  # fold blocks 2,3
    wqf_d = nc.dram_tensor("wqf", [C, 27, C], BF16, kind="ExternalInput")
    woutT_d = nc.dram_tensor("woutT", [256, C], BF16, kind="ExternalInput")
    temp_d = nc.dram_tensor("tempv", [C, 1], F32, kind="ExternalInput")
    mask0_d = nc.dram_tensor("mask0", [C, 1], F32, kind="ExternalInput")
    mask1_d = nc.dram_tensor("mask1", [C, 1], F32, kind="ExternalInput")
    eye_d = nc.dram_tensor("eye96", [C, C], F32, kind="ExternalInput")
    out_d = nc.dram_tensor("out", [C, DD, OWN, W], F32, kind="ExternalOutput")

    LX = HS * WP          # x/y flat length (1568)
    LZ = 14 * WP          # z / v / x1 / y2 frame length (1372)
    LO = OWN * WP         # owned-rows flat length (1176)

    with tile.TileContext(nc) as tc:
        from contextlib import ExitStack
        ctx = ExitStack()
        cpool = ctx.enter_context(tc.tile_pool(name="consts", bufs=1))

        # ---- load constants ----
        def load_const(name, dram, shape, dtype=F32, cast=None):
            t = cpool.tile(shape, dtype, tag=name)
            nc.sync.dma_start(t[:], dram[:])
            if cast is not None:
                tb = cpool.tile(shape, cast, tag=name + "_b")
                nc.vector.tensor_copy(tb[:], t[:])
                return tb
            return t

        wqT_b = load_const("wqT", wqT_d, [C, C], dtype=BF16)
        dwq_s = load_const("dwq", dwq_d, [C, 27])
        wpoT_b = load_const("wpoT", wpoT_d, [C, C], dtype=BF16)
        winT_b = load_const("winT", winT_d, [C, HID2], dtype=BF16)
        ffdw2 = cpool.tile([128, 27], F32, tag="ffdw2")
        ffdw3 = cpool.tile([128, 27], F32, tag="ffdw3")
        nc.sync.dma_start(ffdw2[:], ffdw_d[256:384, :])
        nc.sync.dma_start(ffdw3[:], ffdw_d[384:512, :])
        temp_s = load_const("tempv", temp_d, [C, 1])
        mask0_s = load_const("mask0", mask0_d, [C, 1])
        mask1_s = load_const("mask1", mask1_d, [C, 1])
        eye_s = load_const("eye96", eye_d, [C, C])
        woutT_b0 = cpool.tile([128, C], BF16, tag="woutT0")
        woutT_b1 = cpool.tile([128, C], BF16, tag="woutT1")
        nc.sync.dma_start(woutT_b0[:], woutT_d[0:128, :])
        nc.sync.dma_start(woutT_b1[:], woutT_d[128:256, :])
        # fold weights: bf16 [96, 27*96] / [96, 27*256]
        wkf_b = cpool.tile([C, 27, C], BF16, tag="wkf")
        wvf_b = cpool.tile([C, 27, C], BF16, tag="wvf")
        winf_b = cpool.tile([C, 27, 256], BF16, tag="winf")
        nc.sync.dma_start(wkf_b[:], wkf_d[:])
        nc.sync.dma_start(wvf_b[:], wvf_d[:])
        nc.sync.dma_start(winf_b[:], winf_d[:])
        if a_fold_q:
            wqf_b = cpool.tile([C, 27, C], BF16, tag="wqf")
            nc.sync.dma_start(wqf_b[:], wqf_d[:])
        if b_fold[2] or b_fold[3]:
            winf2_b = cpool.tile([C, 27, 256], BF16, tag="winf2")
            nc.sync.dma_start(winf2_b[:], winf2_d[:])
        ones_b = cpool.tile([C, C], BF16, tag="ones_b")
        nc.vector.memset(ones_b[:], 1.0)
        ones1 = cpool.tile([1, C], F32, tag="ones1")
        nc.vector.memset(ones1[:], 1.0)
        eps_s = cpool.tile([C, 1], F32, tag="eps_s")
        nc.vector.memset(eps_s[:], EPS)

        sqq_acc = cpool.tile([C, DD], F32, tag="sqq_acc")
        sqk_acc = cpool.tile([C, DD], F32, tag="sqk_acc")

        # ---- pools ----
        vpool = ctx.enter_context(tc.tile_pool(name="vpool", bufs=DD))
        ctxA = ExitStack()
        xA = ctxA.enter_context(tc.tile_pool(name="xA", bufs=2))
        sqp = ctxA.enter_context(tc.tile_pool(name="sqp", bufs=2))
        rsp = ctxA.enter_context(tc.tile_pool(name="rsp", bufs=2))
        yp = ctxA.enter_context(tc.tile_pool(name="yp", bufs=3))
        zqp = ctxA.enter_context(tc.tile_pool(name="zqp", bufs=3))
        qtp = ctxA.enter_context(tc.tile_pool(name="qtp", bufs=2))
        qbp = ctxA.enter_context(tc.tile_pool(name="qbp", bufs=2))
        qTp = ctxA.enter_context(tc.tile_pool(name="qTp", bufs=2))
        scr = ctxA.enter_context(tc.tile_pool(name="scr", bufs=2))

        psG = ctxA.enter_context(tc.tile_pool(name="psG", bufs=1, space="PSUM"))
        psA = ctxA.enter_context(tc.tile_pool(name="psA", bufs=4, space="PSUM"))

        G_ps = psG.tile([C, C], F32, tag="G")

        y_tiles, zq_tiles, v_tiles = {}, {}, {}
        gram_first = [True]

        def ln_pass(src_ap, Lf, tag_prefix, pool_s, pool_r, psum_pool, ps_tag="psA"):
            """uncentered biasfree-LN rstd over channels; src [C, Lf] f32 -> rstd bf16 [C, Lf]"""
            xsq = pool_s.tile([C, Lf], BF16, tag=tag_prefix + "sq")
            nc.scalar.activation(xsq[:], src_ap, AF.Square)
            tln = pool_r.tile([C, Lf], BF16, tag=tag_prefix + "ln")
            n_ch = 4
            csz = Lf // n_ch
            for c0, cn in _chunks(Lf, csz):
                q_ps = psum_pool.tile([C, cn], F32, tag=ps_tag)
                nc.tensor.matmul(q_ps[:], ones_b[:], xsq[:, c0:c0 + cn],
                                 start=True, stop=True)
                nc.scalar.activation(tln[:, c0:c0 + cn], q_ps[:], AF.Ln,
                                     bias=eps_s[:], scale=1.0 / C)
            nc.scalar.activation(tln[:], tln[:], AF.Exp, scale=-0.5)
            return tln

        def fold_conv(y_src, wf_b, o_ch, out_row0, out_len, row0_frame, psum_pool,
                      evac):
            """PE-fold dwconv+conv1x1: 27 matmuls per chunk accumulating in psum.
            y_src: dict of y tiles (flat, guard 1); reads at 1+(out_row0+dh)*98+dw+c0.
            evac(c0, cn, psum_tile) consumes each finished chunk."""
            for c0, cn in _chunks(out_len, 490):
                ps = psum_pool.tile([o_ch, cn], F32, tag="psA")
                for t, (dd, dh, dw_) in enumerate(TAPS):
                    yt = y_src[min(max(dcur[0] + dd, 0), DD - 1)]
                    base = 1 + (out_row0 + dh) * WP + dw_ + c0
                    nc.tensor.matmul(ps[:], wf_b[:, t, :], yt[:, base:base + cn],
                                     start=(t == 0), stop=(t == 26))
                evac(c0, cn, ps)

        dcur = [0]

        def a_emit(d):
            dcur[0] = d
            q_bf = qbp.tile([C, OWN, W], BF16, tag="q_bf")
            if a_fold_q:
                def q_evac(c0, cn, ps):
                    rn = cn // WP
                    r0 = c0 // WP
                    p3 = ps[:].rearrange("p (r w) -> p r w", w=WP)
                    nc.scalar.activation(q_bf[:, r0:r0 + rn, :], p3[:, :, 1:97],
                                         AF.Copy)
                fold_conv(y_tiles, wqf_b, C, 2, LO, None, psA, q_evac)
            else:
                # --- q: DVE tap chain on z_q ---
                q_t = qtp.tile([C, LO + 2], BF16, tag="q_t")
                for t, (dd, dh, dw_) in enumerate(TAPS):
                    zt = zq_tiles[min(max(d + dd, 0), DD - 1)]
                    base = 99 + dh * WP + dw_
                    src = zt[:, base:base + LO]
                    if t == 0:
                        nc.vector.tensor_scalar(q_t[:, 0:LO], src, dwq_s[:, 0:1],
                                                None, ALU.mult)
                    else:
                        nc.vector.scalar_tensor_tensor(q_t[:, 0:LO], src,
                                                       dwq_s[:, t:t + 1],
                                                       q_t[:, 0:LO],
                                                       ALU.mult, ALU.add)
                q3 = q_t[:, 0:LO].rearrange("p (r w) -> p r w", w=WP)
                nc.scalar.activation(q_bf[:], q3[:, :, 1:97], AF.Copy)

            # --- k: PE fold (owned rows frame LO) ---
            k_bf = qbp.tile([C, OWN, W], BF16, tag="k_bf")

            def k_evac(c0, cn, ps):
                rn = cn // WP
                r0 = c0 // WP
                p3 = ps[:].rearrange("p (r w) -> p r w", w=WP)
                nc.scalar.activation(k_bf[:, r0:r0 + rn, :], p3[:, :, 1:97], AF.Copy)

            fold_conv(y_tiles, wkf_b, C, 2, LO, None, psA, k_evac)

            # --- v: PE fold (rows 1..14, frame LZ), keep flat bf16 ---
            v_bf = vpool.tile([C, LZ + 3], BF16, tag="v_bf")
            v_tiles[d] = v_bf

            def v_evac(c0, cn, ps):
                nc.scalar.activation(v_bf[:, 1 + c0:1 + c0 + cn], ps[:], AF.Copy)

            fold_conv(y_tiles, wvf_b, C, 1, LZ, None, psA, v_evac)

            # --- sumsq + gram ---
            sc1 = scr.tile([C, OWN * W], BF16, tag="sc1")
            nc.scalar.activation(sc1[:], q_bf[:].rearrange("p r w -> p (r w)"),
                                 AF.Square, accum_out=sqq_acc[:, d:d + 1])
            sc2 = scr.tile([C, OWN * W], BF16, tag="sc2")
            nc.scalar.activation(sc2[:], k_bf[:].rearrange("p r w -> p (r w)"),
                                 AF.Square, accum_out=sqk_acc[:, d:d + 1])
            qT = qTp.tile([128, 9, C], BF16, tag="qT")
            kT = qTp.tile([128, 9, C], BF16, tag="kT")
            nc.sync.dma_start_transpose(qT[:], q_bf[:].rearrange("p r w -> p (r w)"))
            nc.sync.dma_start_transpose(kT[:], k_bf[:].rearrange("p r w -> p (r w)"))
            for j in range(9):
                nc.tensor.matmul(G_ps[:], qT[:, j, :], kT[:, j, :],
                                 start=gram_first[0], stop=(d == DD - 1 and j == 8))
                gram_first[0] = False

        # ================= phase A =================
        for d in range(DD):
            x_t = xA.tile([C, LX + 3], F32, tag="x_t")
            x3 = x_t[:, 1:1 + LX].rearrange("p (r w) -> p r w", w=WP)
            nc.sync.dma_start(x3[:, :, 1:97], x_d[:, d, :, :])
            nc.scalar.activation(x3[:, :, 0:1], x3[:, :, 1:2], AF.Copy)
            nc.scalar.activation(x3[:, :, 97:98], x3[:, :, 96:97], AF.Copy)
            rstd = ln_pass(x_t[:, 1:1 + LX], LX, "A", sqp, rsp, psA)
            y_t = yp.tile([C, LX + 3], BF16, tag="y_t")
            nc.vector.memset(y_t[:, 0:1], 0.0)
            nc.vector.memset(y_t[:, 1 + LX:LX + 3], 0.0)
            nc.vector.tensor_mul(y_t[:, 1:1 + LX], x_t[:, 1:1 + LX], rstd[:])
            y_tiles[d] = y_t
            if not a_fold_q:
                # z_q = wqT^T @ y rows 1..14
                zq = zqp.tile([C, LZ + 3], BF16, tag="zq")
                nc.vector.memset(zq[:, 0:1], 0.0)
                nc.vector.memset(zq[:, 1 + LZ:LZ + 3], 0.0)
                for c0, cn in _chunks(LZ, 490):
                    ps = psA.tile([C, cn], F32, tag="psA")
                    nc.tensor.matmul(ps[:], wqT_b[:],
                                     y_t[:, 1 + WP + c0:1 + WP + c0 + cn],
                                     start=True, stop=True)
                    nc.scalar.activation(zq[:, 1 + c0:1 + c0 + cn], ps[:], AF.Copy)
                zq_tiles[d] = zq
            if d >= 1:
                a_emit(d - 1)
        a_emit(DD - 1)

        # ================= reduce =================
        sqq = cpool.tile([C, 1], F32, tag="sqq")
        sqk = cpool.tile([C, 1], F32, tag="sqk")
        nc.vector.tensor_reduce(sqq[:], sqq_acc[:], X, ALU.add)
        nc.vector.tensor_reduce(sqk[:], sqk_acc[:], X, ALU.add)
        pack = cpool.tile([C, C + 2], F32, tag="pack")
        nc.scalar.activation(pack[:, 0:C], G_ps[:], AF.Copy)
        nc.vector.tensor_copy(pack[:, C:C + 1], sqq[:])
        nc.vector.tensor_copy(pack[:, C + 1:C + 2], sqk[:])

        ctxA.close()
        tot = cpool.tile([C, C + 2], F32, tag="tot")
        if not with_collective:
            nc.vector.tensor_copy(tot[:], pack[:])
        elif reduce_mode == "collective":
            dram = ctx.enter_context(tc.tile_pool(name="dram", bufs=1, space="DRAM"))
            red_in = dram.tile([C, C + 2], F32, tag="red_in")
            red_out = dram.tile([C, C + 2], F32, tag="red_out")
            nc.sync.dma_start(red_in[:], pack[:])
            nc.gpsimd.collective_compute(
                "AllReduce", ALU.add,
                replica_groups=[list(range(num_cores))],
                ins=[red_in.opt()],
                outs=[red_out.opt()],
            )
            nc.sync.dma_start(tot[:], red_out[:])
        else:
            # peer-DMA all-reduce: each core sends its pack to all 7 peers
            # (relative dest Δtpb=j lands in sender-unique slot j), then sums.
            PW = C + 2
            pack128 = cpool.tile([128, PW], F32, tag="pack128")
            slots = cpool.tile([128, (num_cores - 1) * PW], F32, tag="slots")
            acc128 = cpool.tile([128, PW], F32, tag="acc128")
            nc.vector.memset(pack128[96:128, :], 0.0)
            nc.vector.tensor_copy(pack128[0:96, :], pack[:])
            rsem = nc.alloc_semaphore("peer_rsem")
            lsem = nc.alloc_semaphore("peer_lsem")
            with tc.tile_critical():
                g = nc.gpsimd
                for j in range(1, num_cores):
                    rd = [None] * 8
                    rd[j] = (0, j)
                    g.remote_dma_broadcast(slots[:, (j - 1) * PW:j * PW],
                                           pack128[:], remote_sem=rsem,
                                           local_sem=lsem, rdests=rd)
                g.trigger_dma(count=num_cores - 1)
                g.wait_ge(rsem, 2 * (num_cores - 1))
                g.tensor_add(acc128[:], pack128[:], slots[:, 0:PW])
                for j in range(2, num_cores):
                    g.tensor_add(acc128[:], acc128[:],
                                 slots[:, (j - 1) * PW:j * PW])
                g.tensor_copy(tot[:], acc128[0:96, :])

        # ================= attention matrix =================
        psS = ctx.enter_context(tc.tile_pool(name="psS", bufs=1, space="PSUM"))
        nq = cpool.tile([C, 1], F32, tag="nq")
        nk = cpool.tile([C, 1], F32, tag="nk")
        nc.scalar.activation(nq[:], tot[:, C:C + 1], AF.Sqrt)
        nc.scalar.activation(nk[:], tot[:, C + 1:C + 2], AF.Sqrt)
        nc.vector.tensor_scalar_max(nq[:], nq[:], 1e-12)
        nc.vector.tensor_scalar_max(nk[:], nk[:], 1e-12)
        rq = cpool.tile([C, 1], F32, tag="rq")
        rk = cpool.tile([C, 1], F32, tag="rk")
        nc.vector.reciprocal(rq[:], nq[:])
        nc.vector.reciprocal(rk[:], nk[:])
        rqt = cpool.tile([C, 1], F32, tag="rqt")
        nc.vector.tensor_mul(rqt[:], rq[:], temp_s[:])
        rkT_ps = psS.tile([1, C], F32, tag="psS_row")
        nc.tensor.transpose(rkT_ps[:], rk[:], eye_s[:])
        rkT = cpool.tile([1, C], F32, tag="rkT")
        nc.scalar.activation(rkT[:], rkT_ps[:], AF.Copy)
        rk_rep = psS.tile([C, C], F32, tag="psS_rep")
        nc.tensor.matmul(rk_rep[:], ones1[:], rkT[:], start=True, stop=True)
        lg = cpool.tile([C, C], F32, tag="lg")
        nc.vector.scalar_tensor_tensor(lg[:], tot[:, 0:C], rqt[:], rk_rep[:],
                                       ALU.mult, ALU.mult)
        ma = cpool.tile([C, 1], F32, tag="ma")
        mb = cpool.tile([C, 1], F32, tag="mb")
        nc.vector.tensor_reduce(ma[:], lg[:, 0:48], X, ALU.max)
        nc.vector.tensor_reduce(mb[:], lg[:, 48:96], X, ALU.max)
        nc.vector.tensor_scalar_mul(mb[:], mb[:], mask1_s[:])
        mneg = cpool.tile([C, 1], F32, tag="mneg")
        nc.vector.scalar_tensor_tensor(mneg[:], ma[:], mask0_s[:], mb[:],
                                       ALU.mult, ALU.add)
        nc.vector.tensor_scalar_mul(mneg[:], mneg[:], -1.0)
        et = cpool.tile([C, C], F32, tag="et")
        nc.scalar.activation(et[:], lg[:], AF.Exp, bias=mneg[:])
        sa = cpool.tile([C, 1], F32, tag="sa")
        sb = cpool.tile([C, 1], F32, tag="sb")
        nc.vector.tensor_reduce(sa[:], et[:, 0:48], X, ALU.add)
        nc.vector.tensor_reduce(sb[:], et[:, 48:96], X, ALU.add)
        nc.vector.tensor_scalar_mul(sb[:], sb[:], mask1_s[:])
        ssum = cpool.tile([C, 1], F32, tag="ssum")
        nc.vector.scalar_tensor_tensor(ssum[:], sa[:], mask0_s[:], sb[:],
                                       ALU.mult, ALU.add)
        rsum = cpool.tile([C, 1], F32, tag="rsum")
        nc.vector.reciprocal(rsum[:], ssum[:])
        attn = cpool.tile([C, C], F32, tag="attn")
        nc.vector.tensor_scalar(attn[:, 0:48], et[:, 0:48], rsum[:], mask0_s[:],
                                ALU.mult, ALU.mult)
        nc.vector.tensor_scalar(attn[:, 48:96], et[:, 48:96], rsum[:], mask1_s[:],
                                ALU.mult, ALU.mult)
        attnT_ps = psS.tile([C, C], F32, tag="psS_rep")
        nc.tensor.transpose(attnT_ps[:], attn[:], eye_s[:])
        attnT_b = cpool.tile([C, C], BF16, tag="attnT_b")
        nc.scalar.activation(attnT_b[:], attnT_ps[:], AF.Copy)

        # ================= phase B =================
        xB = ctx.enter_context(tc.tile_pool(name="xB", bufs=2))
        oap = ctx.enter_context(tc.tile_pool(name="oap", bufs=2))
        x1p = ctx.enter_context(tc.tile_pool(name="x1p", bufs=3))
        sq2 = ctx.enter_context(tc.tile_pool(name="sq2", bufs=2))
        rs2 = ctx.enter_context(tc.tile_pool(name="rs2", bufs=2))
        y2p = ctx.enter_context(tc.tile_pool(name="y2p", bufs=3))
        zhp = ctx.enter_context(tc.tile_pool(name="zhp", bufs=3))
        hp = ctx.enter_context(tc.tile_pool(name="hp", bufs=2))
        gp = ctx.enter_context(tc.tile_pool(name="gp", bufs=2))
        otp = ctx.enter_context(tc.tile_pool(name="otp", bufs=2))
        psB = ctx.enter_context(tc.tile_pool(name="psB", bufs=4, space="PSUM"))

        x1_tiles, y2_tiles, zh2_tiles, zh3_tiles = {}, {}, {}, {}
        zh_map = {2: zh2_tiles, 3: zh3_tiles}

        def b_emit(d):
            dcur[0] = d
            h_blocks = []
            for b in range(4):
                h_b = hp.tile([128, LO + 2], BF16, tag=f"h{b}")
                if b_fold[b]:
                    wsrc = winf_b if b < 2 else winf2_b
                    wf = wsrc[:, :, 128 * (b % 2):128 * (b % 2 + 1)]
                    for c0, cn in _chunks(LO, 490):
                        ps = psB.tile([128, cn], F32, tag="psB")
                        for t, (dd, dh, dw_) in enumerate(TAPS):
                            yt = y2_tiles[min(max(d + dd, 0), DD - 1)]
                            base = 1 + (1 + dh) * WP + dw_ + c0
                            nc.tensor.matmul(ps[:], wf[:, t, :],
                                             yt[:, base:base + cn],
                                             start=(t == 0), stop=(t == 26))
                        nc.scalar.activation(h_b[:, c0:c0 + cn], ps[:], AF.Copy)
                else:
                    zh_tiles = zh_map[b]
                    dwh = ffdw2 if b == 2 else ffdw3
                    for t, (dd, dh, dw_) in enumerate(TAPS):
                        zt = zh_tiles[min(max(d + dd, 0), DD - 1)]
                        base = 99 + dh * WP + dw_
                        src = zt[:, base:base + LO]
                        if t == 0:
                            nc.vector.tensor_scalar(h_b[:, 0:LO], src, dwh[:, 0:1],
                                                    None, ALU.mult)
                        else:
                            nc.vector.scalar_tensor_tensor(h_b[:, 0:LO], src,
                                                           dwh[:, t:t + 1],
                                                           h_b[:, 0:LO],
                                                           ALU.mult, ALU.add)
                h_blocks.append(h_b)
            # gelu gate: g_i = gelu(h_i) * h_{i+2}
            g_blocks = []
            for i in (0, 1):
                gl = gp.tile([128, LO + 2], BF16, tag=f"gl{i}")
                hin = h_blocks[i][:, 0:LO]
                if not GELU_APPROX:
                    nc.scalar.activation(gl[:, 0:LO], hin, AF.Gelu)
                else:
                    u = gp.tile([128, LO], BF16, tag=f"gu{i}")
                    nc.scalar.activation(u[:], hin, AF.Square)
                    nc.vector.scalar_tensor_tensor(u[:], u[:], 0.044715, hin,
                                                   ALU.mult, ALU.mult)
                    nc.vector.tensor_add(u[:], u[:], hin)
                    nc.scalar.activation(u[:], u[:], AF.Tanh, scale=0.7978845608)
                    nc.vector.scalar_tensor_tensor(u[:], u[:], 0.5, hin,
                                                   ALU.mult, ALU.mult)
                    nc.vector.scalar_tensor_tensor(gl[:, 0:LO], hin, 0.5, u[:],
                                                   ALU.mult, ALU.add)
                nc.vector.tensor_mul(gl[:, 0:LO], gl[:, 0:LO],
                                     h_blocks[2 + i][:, 0:LO])
                g_blocks.append(gl)
            # out conv + residual
            out_t = otp.tile([C, LO + 2], F32, tag="out_t")
            x1 = x1_tiles[d]
            for c0, cn in _chunks(LO, 490):
                ps = psB.tile([C, cn], F32, tag="psB")
                nc.tensor.matmul(ps[:], woutT_b0[:], g_blocks[0][:, c0:c0 + cn],
                                 start=True, stop=False)
                nc.tensor.matmul(ps[:], woutT_b1[:], g_blocks[1][:, c0:c0 + cn],
                                 start=False, stop=True)
                nc.vector.tensor_add(out_t[:, c0:c0 + cn], ps[:],
                                     x1[:, 1 + WP + c0:1 + WP + c0 + cn])
            o3 = out_t[:, 0:LO].rearrange("p (r w) -> p r w", w=WP)
            nc.sync.dma_start(out_d[:, d, :, :], o3[:, :, 1:97])

        for d in range(DD):
            v_bf = v_tiles[d]
            # oa = attnT^T @ v
            oa = oap.tile([C, LZ + 3], BF16, tag="oa")
            for c0, cn in _chunks(LZ, 490):
                ps = psB.tile([C, cn], F32, tag="psB")
                nc.tensor.matmul(ps[:], attnT_b[:], v_bf[:, 1 + c0:1 + c0 + cn],
                                 start=True, stop=True)
                nc.scalar.activation(oa[:, 1 + c0:1 + c0 + cn], ps[:], AF.Copy)
            # x1 = x + wpo^T @ oa
            x2_t = xB.tile([C, LZ + 3], F32, tag="x2_t")
            x23 = x2_t[:, 1:1 + LZ].rearrange("p (r w) -> p r w", w=WP)
            nc.vector.memset(x23[:, :, 0:1], 0.0)
            nc.vector.memset(x23[:, :, 97:98], 0.0)
            nc.sync.dma_start(x23[:, :, 1:97], x_d[:, d, 1:15, :])
            x1 = x1p.tile([C, LZ + 3], F32, tag="x1")
            for c0, cn in _chunks(LZ, 490):
                ps = psB.tile([C, cn], F32, tag="psB")
                nc.tensor.matmul(ps[:], wpoT_b[:], oa[:, 1 + c0:1 + c0 + cn],
                                 start=True, stop=True)
                nc.vector.tensor_add(x1[:, 1 + c0:1 + c0 + cn], ps[:],
                                     x2_t[:, 1 + c0:1 + c0 + cn])
            x13 = x1[:, 1:1 + LZ].rearrange("p (r w) -> p r w", w=WP)
            nc.scalar.activation(x13[:, :, 0:1], x13[:, :, 1:2], AF.Copy)
            nc.scalar.activation(x13[:, :, 97:98], x13[:, :, 96:97], AF.Copy)
            x1_tiles[d] = x1
            # LN2 + y2
            rstd2 = ln_pass(x1[:, 1:1 + LZ], LZ, "B", sq2, rs2, psB, ps_tag="psB")
            y2 = y2p.tile([C, LZ + 3], BF16, tag="y2")
            nc.vector.memset(y2[:, 0:1], 0.0)
            nc.vector.memset(y2[:, 1 + LZ:LZ + 3], 0.0)
            nc.vector.tensor_mul(y2[:, 1:1 + LZ], x1[:, 1:1 + LZ], rstd2[:])
            y2_tiles[d] = y2
            # z_h for DVE blocks
            for b in (2, 3):
                if b_fold[b]:
                    continue
                zh_tiles = zh_map[b]
                zh = zhp.tile([128, LZ + 3], BF16, tag=f"zh{b}")
                nc.vector.memset(zh[:, 0:1], 0.0)
                nc.vector.memset(zh[:, 1 + LZ:LZ + 3], 0.0)
                for c0, cn in _chunks(LZ, 490):
                    ps = psB.tile([128, cn], F32, tag="psB")
                    nc.tensor.matmul(ps[:], winT_b[:, 128 * b:128 * (b + 1)],
                                     y2[:, 1 + c0:1 + c0 + cn], start=True, stop=True)
                    nc.scalar.activation(zh[:, 1 + c0:1 + c0 + cn], ps[:], AF.Copy)
                zh_tiles[d] = zh
            if d >= 1:
                b_emit(d - 1)
        b_emit(DD - 1)

        ctx.close()

    nc.compile()
    return nc


def _prep_weights(inp):
    import ml_dtypes
    bf = ml_dtypes.bfloat16
    w = {}
    ln1 = np.asarray(inp['ln1_w'], np.float32)
    ln2 = np.asarray(inp['ln2_w'], np.float32)
    w_qkv = np.asarray(inp['w_qkv'], np.float32) * ln1[None, :]
    w_dw = np.asarray(inp['w_dw'], np.float32).reshape(288, 27)
    w['wqT'] = np.ascontiguousarray(w_qkv[0:96].T)
    w['dwq'] = np.ascontiguousarray(w_dw[0:96])
    wqT0 = w_qkv[0:96].T
    dwq0 = w_dw[0:96]
    w['wqf'] = np.ascontiguousarray(
        np.stack([wqT0 * dwq0[None, :, t] for t in range(27)]).transpose(1, 0, 2))
    wkT = w_qkv[96:192].T
    dwk = w_dw[96:192]
    w['wkf'] = np.ascontiguousarray(
        np.stack([wkT * dwk[None, :, t] for t in range(27)]).transpose(1, 0, 2))
    wvT = w_qkv[192:288].T
    dwv = w_dw[192:288]
    w['wvf'] = np.ascontiguousarray(
        np.stack([wvT * dwv[None, :, t] for t in range(27)]).transpose(1, 0, 2))
    w['wpoT'] = np.ascontiguousarray(np.asarray(inp['w_po'], np.float32).T)
    w_in = np.asarray(inp['w_in'], np.float32) * ln2[None, :]
    w_in_p = np.zeros((HID2, 96), np.float32)
    w_in_p[0:255] = w_in[0:255]
    w_in_p[256:511] = w_in[255:510]
    w_ffdw = np.asarray(inp['w_ffdw'], np.float32).reshape(510, 27)
    w_ffdw_p = np.zeros((HID2, 27), np.float32)
    w_ffdw_p[0:255] = w_ffdw[0:255]
    w_ffdw_p[256:511] = w_ffdw[255:510]
    w['winT'] = np.ascontiguousarray(w_in_p.T)
    w['ffdw'] = w_ffdw_p
    w['winf'] = np.ascontiguousarray(
        np.stack([w_in_p.T[:, 0:256] * w_ffdw_p[None, 0:256, t].reshape(1, 256)
                  for t in range(27)]).transpose(1, 0, 2))
    w['winf2'] = np.ascontiguousarray(
        np.stack([w_in_p.T[:, 256:512] * w_ffdw_p[None, 256:512, t].reshape(1, 256)
                  for t in range(27)]).transpose(1, 0, 2))
    w_out = np.asarray(inp['w_out'], np.float32)
    w_out_p = np.zeros((96, 256), np.float32)
    w_out_p[:, 0:255] = w_out
    w['woutT'] = np.ascontiguousarray(w_out_p.T)
    w['tempv'] = np.repeat(
        np.asarray(inp['temperature'], np.float32).reshape(2), 48)[:, None].copy()
    mask0 = np.zeros((96, 1), np.float32)
    mask0[0:48] = 1
    w['mask0'] = mask0
    w['mask1'] = np.ascontiguousarray(1 - mask0)
    w['eye96'] = np.eye(96, dtype=np.float32)
    for name in ('wqT', 'wkf', 'wvf', 'wpoT', 'winT', 'winf', 'winf2', 'wqf', 'woutT'):
        w[name] = w[name].astype(bf)
    return w


_CACHE = {}


def kernel(**inputs):
    if 'nc' not in _CACHE:
        _CACHE['nc'] = build_module()
    nc = _CACHE['nc']
    w = _prep_weights(inputs)
    x = np.asarray(inputs['x'], np.float32)[0]     # [96, 16, 96, 96]
    in_maps = []
    for c in range(NCORES):
        h0 = OWN * c
        idx = np.clip(np.arange(h0 - 2, h0 + 14), 0, H - 1)
        m = {'x': np.ascontiguousarray(x[:, :, idx, :])}
        m.update(w)
        in_maps.append(m)
    res = run_bass_kernel_spmd(nc, in_maps, core_ids=list(range(NCORES)))
    out = np.concatenate([res.results[c]['out'] for c in range(NCORES)], axis=2)
    return out[None].astype(np.float32)


if __name__ == "__main__":
    import jax
    out = kernel(**{k: np.asarray(v) for k, v in __import__('reference').setup_inputs().items()})
    print(out.shape, out.dtype)

